# revision 27
# baseline (speedup 1.0000x reference)
"""Trainium2 Bass kernel for nn_CustomAttnProcessor (dense transformer block).

Data-parallel over batch B=8 across 8 NeuronCores; one batch element per core.

Per-core dataflow (channel-major activations: [feature_partition, token_free]):
  xT = concat(hiddenT, obj @ linear_w)            [1280, 1056pad]
  ln1T = LN(xT) -> fp8                            -> masked self-attention
      QKV projections in fp8 DoubleRow (2 k-planes per matmul);
      sim computed TRANSPOSED ([key, query]) so softmax probs are directly
      usable as the moving operand of the attn@V matmul; no max-subtraction
      (values are small), denominator via an appended ones-column on V.
  hsT = hiddenT + tanh(a_attn)*attn[:1024]        (tanh folded into weights)
  hsT += tanh(a_dense)*GEGLU_FFN(LN(hsT))         (fp8 DoubleRow FFN)
  out = cross_attention(LN(hsT), enc)             (bf16) token-major output
"""

import os
import sys

import numpy as np
import ml_dtypes

sys.path.insert(0, "/opt/trn_rl_repo")

import concourse.bass as bass
import concourse.tile as tile
from concourse import bacc, mybir
from concourse.bass_utils import run_bass_kernel_spmd

F32 = mybir.dt.float32
F32R = mybir.dt.float32r
BF16 = mybir.dt.bfloat16
FP8 = mybir.dt.float8e4
U8 = mybir.dt.uint8
AF = mybir.ActivationFunctionType
ALU = mybir.AluOpType
DR = mybir.MatmulPerfMode.DoubleRow

B = 8
NV = 1024          # visual tokens
NOBJ = 30
N = NV + NOBJ      # 1054
NP = 1056          # padded token count (free dim)
NJC = 9            # key-dim 128-chunks over NP (last chunk = 32 rows)
D = 1280
KD = D // 128      # 10
DTXT = 768
KT = DTXT // 128   # 6
LTXT = 77
LTP = 78           # padded (fp32r needs even moving dims)
HC, CC = 8, 64     # masked self-attention heads
HA, CA = 20, 64    # cross-attention heads
INNER_C = HC * CC  # 512
INNER_A = HA * CA  # 1280
DFF = 4 * D        # 5120
KF = DFF // 128    # 40
EPS = 1e-5
SCALE = CC ** -0.5  # 0.125
WS = 32.0          # fp8 weight pre-scale (host); 1/WS folded into psum drain
FS = 8.0           # ffT (a*gelu) fp8 pre-scale

IC_NP = [(0, 512), (512, 512), (1024, 32)]   # token chunks for 1056
IC_NV = [(0, 512), (512, 512)]               # token chunks for 1024
DC_D = [(0, 512), (512, 512), (1024, 256)]   # feature chunks for 1280


class LNPipe:
    """LN over the partition (feature) axis of channel-major x, split so the
    stats matmuls can be emitted inline with the producer of x (keeping the
    PE busy across phase transitions).

    Stats via ones-matmul (cross-partition reduce); mean/rstd rows broadcast
    back across partitions with a rank-1 (K=1) ones matmul into PSUM.
    """

    def __init__(self, tc, nc, name, n_tok, kc_n, ones_r):
        self.tc, self.nc, self.n_tok, self.kc_n = tc, nc, n_tok, kc_n
        self.ones_r = ones_r
        self.chunks = [(o, min(w, n_tok - o)) for (o, w) in IC_NP if o < n_tok]
        # alloc order matters: released LIFO at the end of finish()
        self.stat_ps = tc.alloc_tile_pool(name=f"{name}_st", bufs=1, space="PSUM")
        self.sqp = tc.alloc_tile_pool(name=f"{name}_sq", bufs=2)
        self.rows = tc.alloc_tile_pool(name=f"{name}_rows", bufs=1)
        self.bcast = tc.alloc_tile_pool(name=f"{name}_bc", bufs=1)
        self.tmps = tc.alloc_tile_pool(name=f"{name}_tmp", bufs=2)
        self.ps_mu = [self.stat_ps.tile([1, w], F32, tag=f"ps_mu{i}",
                                        name=f"{name}_mu{i}")
                      for i, (o, w) in enumerate(self.chunks)]
        self.ps_ex = [self.stat_ps.tile([1, w], F32, tag=f"ps_ex{i}",
                                        name=f"{name}_ex{i}")
                      for i, (o, w) in enumerate(self.chunks)]

    def stats(self, x, kc, ci, start, stop):
        """Emit stats for x[:, kc, chunk ci]."""
        nc = self.nc
        o, w = self.chunks[ci]
        sq = self.sqp.tile([128, w], F32R, tag=f"ln_sq{ci}", name=f"sq{ci}")
        nc.vector.tensor_mul(sq[:], x[:, kc, o:o + w].bitcast(F32),
                             x[:, kc, o:o + w].bitcast(F32))
        nc.tensor.matmul(self.ps_mu[ci][:], self.ones_r[:], x[:, kc, o:o + w],
                         start=start, stop=stop)
        nc.tensor.matmul(self.ps_ex[ci][:], self.ones_r[:], sq[:, :],
                         start=start, stop=stop)

    def finish(self, x, out, g_tile, b_tile, ones_bc, eps_t, d_feat=D):
        tc, nc, n_tok = self.tc, self.nc, self.n_tok
        rows, bcast, tmps = self.rows, self.bcast, self.tmps
        mu_row = rows.tile([1, n_tok], F32R, tag="mu_row")
        ex_row = rows.tile([1, n_tok], F32, tag="ex_row")
        t_row = rows.tile([1, n_tok], F32, tag="t_row")
        var_row = rows.tile([1, n_tok], F32, tag="var_row")
        rs_row = rows.tile([1, n_tok], F32R, tag="rs_row")
        inv_d = 1.0 / float(d_feat)
        for i, (o, w) in enumerate(self.chunks):
            nc.scalar.activation(mu_row[:, o:o + w], self.ps_mu[i][:],
                                 AF.Copy, scale=inv_d)
            nc.scalar.activation(ex_row[:, o:o + w], self.ps_ex[i][:],
                                 AF.Copy, scale=inv_d)
        nc.vector.tensor_mul(t_row[:], mu_row[:].bitcast(F32),
                             mu_row[:].bitcast(F32))
        nc.vector.tensor_sub(var_row[:], ex_row[:], t_row[:])
        self.stat_ps.release()
        nc.scalar.activation(t_row[:], var_row[:], AF.Sqrt, bias=eps_t[:])
        nc.vector.reciprocal(rs_row[:], t_row[:])
        with tc.tile_pool(name="ln_bps", bufs=1, space="PSUM") as bps:
            mu_b = bcast.tile([128, n_tok], F32, tag="mu_b")
            rs_b = bcast.tile([128, n_tok], F32, tag="rs_b")
            for i, (o, w) in enumerate(self.chunks):
                pmu = bps.tile([128, w], F32, tag="pmu", name=f"pmu{i}")
                prs = bps.tile([128, w], F32, tag="prs", name=f"prs{i}")
                nc.tensor.matmul(pmu[:], ones_bc[:], mu_row[:, o:o + w],
                                 start=True, stop=True)
                nc.tensor.matmul(prs[:], ones_bc[:], rs_row[:, o:o + w],
                                 start=True, stop=True)
                nc.scalar.activation(mu_b[:, o:o + w], pmu[:], AF.Copy)
                nc.scalar.activation(rs_b[:, o:o + w], prs[:], AF.Copy)
        for kc in range(self.kc_n):
            t1 = tmps.tile([128, n_tok], F32, tag="ln_t1")
            nc.vector.tensor_sub(t1[:], x[:, kc, :].bitcast(F32), mu_b[:])
            nc.vector.tensor_mul(t1[:], t1[:], rs_b[:])
            nc.vector.tensor_scalar(out=out[:, kc, :], in0=t1[:],
                                    scalar1=g_tile[:, kc:kc + 1],
                                    scalar2=b_tile[:, kc:kc + 1],
                                    op0=ALU.mult, op1=ALU.add)
        self.tmps.release()
        self.bcast.release()
        self.rows.release()
        self.sqp.release()


def build_nc():
    nc = bacc.Bacc("TRN2", target_bir_lowering=False, debug=False, num_devices=B)

    # ---- DRAM I/O (per core) ----
    d_hidT = nc.dram_tensor("hidT", [D, NV], F32R, kind="ExternalInput").ap()
    d_objT = nc.dram_tensor("objT", [DTXT, NOBJ], F32R, kind="ExternalInput").ap()
    d_encT = nc.dram_tensor("encT", [DTXT, LTP], BF16, kind="ExternalInput").ap()
    d_maskT = nc.dram_tensor("maskT", [HC, NP, NP], BF16, kind="ExternalInput").ap()
    d_wlin = nc.dram_tensor("w_lin", [DTXT, D], F32R, kind="ExternalInput").ap()
    d_blin = nc.dram_tensor("b_lin", [D], F32, kind="ExternalInput").ap()
    d_wq = nc.dram_tensor("w_q", [D, INNER_C], FP8, kind="ExternalInput").ap()
    d_wk = nc.dram_tensor("w_k", [D, INNER_C], FP8, kind="ExternalInput").ap()
    d_wv = nc.dram_tensor("w_v", [D, INNER_C], FP8, kind="ExternalInput").ap()
    d_wco = nc.dram_tensor("w_co", [INNER_C, D], BF16, kind="ExternalInput").ap()
    d_bco = nc.dram_tensor("b_co", [D], F32, kind="ExternalInput").ap()
    d_wg = nc.dram_tensor("w_geglu", [D, 2 * DFF], FP8, kind="ExternalInput").ap()
    d_bg = nc.dram_tensor("b_geglu", [2 * DFF], F32, kind="ExternalInput").ap()
    d_wf = nc.dram_tensor("w_ffout", [DFF, D], FP8, kind="ExternalInput").ap()
    d_bf = nc.dram_tensor("b_ffout", [D], F32, kind="ExternalInput").ap()
    d_wqa = nc.dram_tensor("w_qa", [D, INNER_A], BF16, kind="ExternalInput").ap()
    d_wka = nc.dram_tensor("w_ka", [DTXT, INNER_A], BF16, kind="ExternalInput").ap()
    d_wva = nc.dram_tensor("w_va", [DTXT, INNER_A], BF16, kind="ExternalInput").ap()
    d_woa = nc.dram_tensor("w_oa", [INNER_A, D], BF16, kind="ExternalInput").ap()
    d_boa = nc.dram_tensor("b_oa", [D], F32, kind="ExternalInput").ap()
    d_norm = nc.dram_tensor("norms", [6, D], F32, kind="ExternalInput").ap()
    d_vones = nc.dram_tensor("vones", [LTP, HA], BF16, kind="ExternalInput").ap()
    d_out = nc.dram_tensor("out", [NV, D], F32, kind="ExternalOutput").ap()

    r128 = lambda ap: ap.rearrange("(kc p) n -> p kc n", p=128)

    with tile.TileContext(nc) as tc, \
            nc.allow_low_precision(reason="fp8/bf16 rounding is intentional"):
        cst = tc.alloc_tile_pool(name="cst", bufs=1)
        ones_f = cst.tile([128, 128], F32, tag="ones_f")  # memset can't write f32r
        nc.vector.memset(ones_f[:], 1.0)
        ones_r = cst.tile([128, 1], F32R, tag="ones_r")
        nc.vector.tensor_copy(ones_r[:], ones_f[:, 0:1])
        eps_t = cst.tile([1, 1], F32, tag="eps")
        nc.vector.memset(eps_t[:], EPS)
        ones_bc = cst.tile([1, 128], F32R, tag="ones_bc")
        nc.vector.tensor_copy(ones_bc[:], ones_f[0:1, :])
        ones_b16 = cst.tile([128, 256], BF16, tag="ones_b16")
        nc.vector.memset(ones_b16[:], 1.0)
        zeros_f = cst.tile([128, KD, 2], F32, tag="zeros_f")
        nc.vector.memset(zeros_f[:], 0.0)
        norm_t = cst.tile([128, 6, KD], F32, tag="norms")
        nc.sync.dma_start(out=norm_t[:], in_=d_norm.rearrange("g (kc p) -> p g kc", p=128))
        blin_t = cst.tile([128, KD], F32, tag="blin")
        nc.sync.dma_start(out=blin_t[:], in_=d_blin.rearrange("(kc p) -> p kc", p=128))
        bco_t = cst.tile([128, KD], F32, tag="bco")
        nc.sync.dma_start(out=bco_t[:], in_=d_bco.rearrange("(kc p) -> p kc", p=128))
        bg_t = cst.tile([128, 2 * DFF // 128], F32, tag="bg")
        nc.sync.dma_start(out=bg_t[:], in_=d_bg.rearrange("(kc p) -> p kc", p=128))
        bf_t = cst.tile([128, KD], F32, tag="bf")
        nc.sync.dma_start(out=bf_t[:], in_=d_bf.rearrange("(kc p) -> p kc", p=128))
        boa_b = cst.tile([128, D], F32, tag="boa_b")
        nc.sync.dma_start(out=boa_b[:], in_=bass.AP(
            tensor=d_boa.tensor, offset=d_boa.offset, ap=[[0, 128]] + d_boa.ap))

        res = tc.alloc_tile_pool(name="res", bufs=1)  # hsT: lives phases 1-3
        hsT = res.tile([128, KD, NV], F32R, tag="hsT")

        # ================= Phase 1: concat + LN1 + masked self-attention ==========
        pps = tc.alloc_tile_pool(name="pps", bufs=2, space="PSUM")  # drain psums
        px = tc.alloc_tile_pool(name="px", bufs=1)
        xT = px.tile([128, KD, NP], F32R, tag="xT")
        nc.vector.tensor_copy(xT[:, :, N:NP], zeros_f[:])
        ln1 = LNPipe(tc, nc, "ln1", NP, KD, ones_r)
        for kc in range(KD):
            nc.sync.dma_start(out=xT[:, kc, 0:NV], in_=r128(d_hidT)[:, kc, :])
            ln1.stats(xT, kc, 0, start=(kc == 0), stop=(kc == KD - 1))
            ln1.stats(xT, kc, 1, start=(kc == 0), stop=(kc == KD - 1))
        obj_sb = px.tile([128, KT, NOBJ], F32R, tag="obj_sb")
        nc.sync.dma_start(out=obj_sb[:], in_=r128(d_objT))
        with tc.tile_pool(name="pwlin", bufs=1) as pwlin:
            wlin = pwlin.tile([128, KT, D], F32R, tag="wlin")
            nc.sync.dma_start(out=wlin[:], in_=r128(d_wlin))
            for mc in range(KD):
                ps = pps.tile([128, NOBJ], F32, tag="ps_proj")
                for kc in range(KT):
                    nc.tensor.matmul(ps[:], wlin[:, kc, mc * 128:(mc + 1) * 128],
                                     obj_sb[:, kc, :], start=(kc == 0), stop=(kc == KT - 1))
                nc.scalar.activation(xT[:, mc, NV:N], ps[:], AF.Identity,
                                     bias=blin_t[:, mc:mc + 1])
                ln1.stats(xT, mc, 2, start=(mc == 0), stop=(mc == KD - 1))

        # ln1T only feeds QKV; fp8 is enough (attention residual is tanh(0.1)-scaled).
        pln1 = tc.alloc_tile_pool(name="pln1", bufs=1, side="right")
        ln1T = pln1.tile([128, KD, NP], FP8, tag="ln1T")
        ln1.finish(xT, ln1T, norm_t[:, 0, :], norm_t[:, 1, :], ones_bc, eps_t)
        px.release()  # xT dead; residual re-reads hiddenT from DRAM

        pqk = tc.alloc_tile_pool(name="pqk", bufs=1)
        pv1 = tc.alloc_tile_pool(name="pv1", bufs=1)
        qT = pqk.tile([128, 4, NP], BF16, tag="qT")
        # kTz: block-diagonal K — head h's 64 channels live at partitions
        # (h%2)*64, the other 64 partitions are zero, so sim matmuls run as
        # full 128x128 tiles (1 cycle/row) with the full-height q as moving.
        kTz = pqk.tile([128, HC, NP], BF16, tag="kTz")
        nc.vector.memset(kTz[:], 0.0)
        # v1z: V padded to 128 stationary columns: [v(64) | ones | zeros(63)]
        v1z = pv1.tile([128, NJC, HC, 128], BF16, tag="v1z")
        nc.vector.memset(v1z[:, :, :, CC + 1:128], 0.0)
        nc.vector.tensor_copy(v1z[:, :, :, CC],
                              ones_f[:, 0:NJC * HC].rearrange("p (a b) -> p a b", a=NJC))
        with tc.tile_pool(name="pwcma", bufs=2) as pwcma:
            for d_w, dest in ((d_wq, qT), (d_wk, kTz)):
                for mc2 in range(2):
                    w_t = pwcma.tile([128, KD, 256], FP8, tag="w_cma")
                    nc.sync.dma_start(out=w_t[:],
                                      in_=r128(d_w[:, mc2 * 256:(mc2 + 1) * 256]))
                    for mh in range(2):
                        mc = mc2 * 2 + mh
                        for (io, iw) in IC_NP:
                            ps = pps.tile([128, iw], F32, tag="ps_proj")
                            for kc in range(0, KD, 2):
                                nc.tensor.matmul(ps[:],
                                                 w_t[:, kc:kc + 2, mh * 128:(mh + 1) * 128],
                                                 ln1T[:, kc:kc + 2, io:io + iw],
                                                 start=(kc == 0), stop=(kc == KD - 2),
                                                 perf_mode=DR)
                            if dest is qT:
                                nc.scalar.activation(qT[:, mc, io:io + iw], ps[:],
                                                     AF.Copy, scale=1.0 / WS)
                            else:
                                nc.scalar.activation(kTz[0:64, 2 * mc, io:io + iw],
                                                     ps[0:64, :], AF.Copy,
                                                     scale=1.0 / WS)
                                nc.scalar.activation(kTz[64:128, 2 * mc + 1, io:io + iw],
                                                     ps[64:128, :], AF.Copy,
                                                     scale=1.0 / WS)
            for mc2 in range(2):
                w_t = pwcma.tile([128, KD, 256], FP8, tag="w_cma")
                nc.sync.dma_start(out=w_t[:], in_=r128(d_wv[:, mc2 * 256:(mc2 + 1) * 256]))
                for jc in range(NJC):
                    jw = 128 if jc < NJC - 1 else NP - 128 * (NJC - 1)
                    ps = pps.tile([128, 256], F32, tag="ps_proj")
                    for kc in range(0, KD, 2):
                        nc.tensor.matmul(ps[:jw, :],
                                         ln1T[:, kc:kc + 2, jc * 128:jc * 128 + jw],
                                         w_t[:, kc:kc + 2, :],
                                         start=(kc == 0), stop=(kc == KD - 2),
                                         perf_mode=DR)
                    nc.scalar.activation(v1z[:jw, jc, mc2 * 4:(mc2 + 1) * 4, 0:CC],
                                         ps[:jw, :].rearrange("p (h c) -> p h c", c=CC),
                                         AF.Copy, scale=1.0 / WS)
        pln1.release()
        pps.release()

        # Attention: simT[j,i] per head; P^T = exp(simT)*maskT; AV via ones-col on V.
        pcat = tc.alloc_tile_pool(name="pcat", bufs=1, side="right")
        catT = pcat.tile([128, 4, NP], BF16, tag="catT")
        pden = tc.alloc_tile_pool(name="pden", bufs=1)
        dens = pden.tile([HC, NP], F32, tag="dens")
        den0 = [pden.tile([1, NP], F32, tag=f"den0_{h}", name=f"den0_{h}")
                for h in range(HC)]
        with (
            tc.tile_pool(name="p1s", bufs=3) as p1s,
            tc.tile_pool(name="psim", bufs=3, space="PSUM") as psim,
            tc.tile_pool(name="pav", bufs=1, space="PSUM") as pav,
            tc.tile_pool(name="pwarm", bufs=1, space="PSUM") as pwarm,
        ):
            warm_ps = pwarm.tile([128, 512], F32, tag="warm_ps")

            def warm(n=2):
                # always-ready matmuls keep the PE clock ramped (p-state)
                for _ in range(n):
                    nc.tensor.matmul(warm_ps[:, 0:256], ones_b16[:, 0:128],
                                     ones_b16[:, :], start=True, stop=True,
                                     skip_group_check=True)
            # Software-pipelined with lookahead 1: sim(i+1) is emitted BEFORE
            # AV(i) so the in-order PE queue never blocks on the exp->mask
            # chain of unit i while sim work is available.
            av_pools = {}

            def emit_sim(h, jc, i):
                pr, hc = (h % 2) * 64, h // 2
                io, iw = IC_NP[i]
                jw = 128 if jc < NJC - 1 else NP - 128 * (NJC - 1)
                if i == 0:
                    m8 = p1s.tile([128, NP], BF16, tag="m8")
                    nc.sync.dma_start(out=m8[:jw, :],
                                      in_=d_maskT[h, jc * 128:jc * 128 + jw, :])
                    emit_sim.m8 = m8
                if jc == 0 and i == 0:
                    av_pools[h] = [
                        pav.tile([128, w], F32, tag=f"ps_av{k}", name=f"ps_av{k}")
                        for k, (o, w) in enumerate(IC_NP)]
                ps_s = psim.tile([128, iw], F32, tag="ps_sim")
                nc.tensor.matmul(ps_s[:jw, :],
                                 kTz[:, h, jc * 128:jc * 128 + jw],
                                 qT[:, hc, io:io + iw],
                                 start=True, stop=True)
                pt = p1s.tile([128, iw], BF16, tag="pt")
                nc.scalar.activation(pt[:jw, :], ps_s[:jw, :], AF.Exp, scale=SCALE)
                ptm = p1s.tile([128, iw], BF16, tag="ptm")
                nc.vector.tensor_mul(ptm[:jw, :], pt[:jw, :],
                                     emit_sim.m8[:jw, io:io + iw])
                return (h, jc, i, jw, ptm)

            def emit_av(st):
                h, jc, i, jw, ptm = st
                pr, hc = (h % 2) * 64, h // 2
                nc.tensor.matmul(av_pools[h][i][:], v1z[:jw, jc, h, :], ptm[:jw, :],
                                 start=(jc == 0), stop=(jc == NJC - 1))
                if jc == NJC - 1:
                    if i == 0:
                        emit_av.den_h = p1s.tile([1, NP], F32, tag="den_h")
                    io, iw = IC_NP[i]
                    nc.vector.tensor_copy(catT[pr:pr + 64, hc, io:io + iw],
                                          av_pools[h][i][0:CC, :])
                    nc.scalar.activation(emit_av.den_h[:, io:io + iw],
                                         av_pools[h][i][CC:CC + 1, :], AF.Copy)
                    if i == len(IC_NP) - 1:
                        # engines can't write partition h directly; DMA the row
                        nc.sync.dma_start(out=dens[h:h + 1, :], in_=emit_av.den_h[:])

            units = [(h, jc, i) for h in range(HC) for jc in range(NJC)
                     for i in range(len(IC_NP))]
            from collections import deque
            pend = deque()
            for (h, jc, i) in units:
                pend.append(emit_sim(h, jc, i))
                warm(2)
                if len(pend) > 2:
                    emit_av(pend.popleft())
            while pend:
                emit_av(pend.popleft())
        # One batched reciprocal for all heads, then per-head row DMA to
        # partition 0 for the rank-1 broadcast matmul.
        nc.vector.reciprocal(dens[:], dens[:])
        with tc.tile_pool(name="pdbc", bufs=2, space="PSUM") as pdbc:
            for h in range(HC):
                pr = (h % 2) * 64
                hc = h // 2
                nc.sync.dma_start(out=den0[h][:], in_=dens[h:h + 1, :])
                for i, (io, iw) in enumerate(IC_NP):
                    pd = pdbc.tile([128, iw], F32, tag=f"den_bc{i}",
                                   name=f"den_bc{i}")
                    nc.tensor.matmul(pd[:], ones_bc[:],
                                     den0[h][:, io:io + iw].bitcast(F32R),
                                     start=True, stop=True)
                    nc.vector.tensor_mul(catT[pr:pr + 64, hc, io:io + iw],
                                         catT[pr:pr + 64, hc, io:io + iw],
                                         pd[pr:pr + 64, :])
        pden.release()
        pv1.release()
        pqk.release()

        # Output projection (tanh(alpha_attn) pre-folded) + residual into hsT.
        # LN2 stats are emitted inline as each hsT chunk lands.
        pln2 = tc.alloc_tile_pool(name="pln2", bufs=1)
        ln2T = pln2.tile([128, KD, NV], FP8, tag="ln2T")
        ln2 = LNPipe(tc, nc, "ln2", NV, KD, ones_r)
        with (
            tc.tile_pool(name="pwco", bufs=1) as pwco,
            tc.tile_pool(name="phid", bufs=3) as phid,
            tc.tile_pool(name="pco", bufs=2, space="PSUM") as pco,
        ):
            w_co = pwco.tile([128, 4, D], BF16, tag="w_co")
            nc.sync.dma_start(out=w_co[:], in_=r128(d_wco))
            for mc in range(KD):
                for ci, (io, iw) in enumerate(IC_NV):
                    ps = pco.tile([128, iw], F32, tag="ps_co")
                    for kc in range(4):
                        nc.tensor.matmul(ps[:], w_co[:, kc, mc * 128:(mc + 1) * 128],
                                         catT[:, kc, io:io + iw],
                                         start=(kc == 0), stop=(kc == 3))
                    hid_r = phid.tile([128, iw], F32R, tag="hid_r")
                    nc.sync.dma_start(out=hid_r[:], in_=r128(d_hidT)[:, mc, io:io + iw])
                    nc.vector.scalar_tensor_tensor(
                        out=hsT[:, mc, io:io + iw], in0=ps[:],
                        scalar=bco_t[:, mc:mc + 1], in1=hid_r[:].bitcast(F32),
                        op0=ALU.add, op1=ALU.add)
                    ln2.stats(hsT, mc, ci, start=(mc == 0), stop=(mc == KD - 1))
        pcat.release()

        # ================= Phase 2: LN2 + GEGLU FFN (fp8 DoubleRow) ==============
        ln2.finish(hsT, ln2T, norm_t[:, 2, :], norm_t[:, 3, :], ones_bc, eps_t)
        pff = tc.alloc_tile_pool(name="pff", bufs=1, side="right")
        ffT = pff.tile([128, KF, NV], FP8, tag="ffT")
        with (
            tc.tile_pool(name="pwg", bufs=2) as pwg,
            tc.tile_pool(name="p2s", bufs=3) as p2s,
            tc.tile_pool(name="p2ps", bufs=2, space="PSUM") as p2ps,
        ):
            for m in range(KF):
                if m % 2 == 0:
                    wga = pwg.tile([128, KD, 256], FP8, tag="wga")
                    nc.sync.dma_start(out=wga[:], in_=r128(d_wg[:, m * 128:(m + 2) * 128]))
                    wgg = pwg.tile([128, KD, 256], FP8, tag="wgg")
                    nc.sync.dma_start(out=wgg[:],
                                      in_=r128(d_wg[:, DFF + m * 128:DFF + (m + 2) * 128]))
                mo = (m % 2) * 128
                gelu_sb = p2s.tile([128, NV], BF16, tag="gelu_sb")
                a_sb = p2s.tile([128, NV], BF16, tag="a_sb")
                for i, (io, iw) in enumerate(IC_NV):
                    ps_a = p2ps.tile([128, iw], F32, tag=f"ps_a{i}", name=f"ps_a{i}")
                    ps_g = p2ps.tile([128, iw], F32, tag=f"ps_g{i}", name=f"ps_g{i}")
                    for kc in range(0, KD, 2):
                        nc.tensor.matmul(ps_a[:], wga[:, kc:kc + 2, mo:mo + 128],
                                         ln2T[:, kc:kc + 2, io:io + iw],
                                         start=(kc == 0), stop=(kc == KD - 2),
                                         perf_mode=DR)
                        nc.tensor.matmul(ps_g[:], wgg[:, kc:kc + 2, mo:mo + 128],
                                         ln2T[:, kc:kc + 2, io:io + iw],
                                         start=(kc == 0), stop=(kc == KD - 2),
                                         perf_mode=DR)
                    # gelu_sb = Gelu(ps_g/WS + bg_gate)
                    nc.scalar.activation(gelu_sb[:, io:io + iw], ps_g[:], AF.Gelu,
                                         bias=bg_t[:, KF + m:KF + m + 1], scale=1.0 / WS)
                    # a_sb = FS*(ps_a/WS + bg_a)  (bias pre-multiplied by FS on host)
                    nc.scalar.activation(a_sb[:, io:io + iw], ps_a[:], AF.Identity,
                                         bias=bg_t[:, m:m + 1], scale=FS / WS)
                    nc.vector.tensor_mul(ffT[:, m, io:io + iw], a_sb[:, io:io + iw],
                                         gelu_sb[:, io:io + iw])
        pln2.release()
        # ffout (tanh(alpha_dense) pre-folded) + residual in place.
        # LN3 stats are emitted inline as each hsT chunk lands.
        ln3 = LNPipe(tc, nc, "ln3", NV, KD, ones_r)
        with (
            tc.tile_pool(name="pwf", bufs=2) as pwf,
            tc.tile_pool(name="pfs", bufs=2) as pfs,
            tc.tile_pool(name="pfps", bufs=3, space="PSUM") as pfps,
        ):
            for mc in range(KD):
                if mc % 2 == 0:
                    wf = pwf.tile([128, KF, 256], FP8, tag="wf")
                    nc.sync.dma_start(out=wf[:], in_=r128(d_wf[:, mc * 128:(mc + 2) * 128]))
                mo = (mc % 2) * 128
                for ci, (io, iw) in enumerate(IC_NV):
                    ps = pfps.tile([128, iw], F32, tag="ps_f")
                    for kc in range(0, KF, 2):
                        nc.tensor.matmul(ps[:], wf[:, kc:kc + 2, mo:mo + 128],
                                         ffT[:, kc:kc + 2, io:io + iw],
                                         start=(kc == 0), stop=(kc == KF - 2),
                                         perf_mode=DR)
                    # tmp = ps/(WS*FS) + bf ; hsT += tmp
                    tmp = pfs.tile([128, iw], F32, tag="fftmp")
                    nc.scalar.activation(tmp[:], ps[:], AF.Identity,
                                         bias=bf_t[:, mc:mc + 1], scale=1.0 / (WS * FS))
                    nc.vector.tensor_add(hsT[:, mc, io:io + iw],
                                         hsT[:, mc, io:io + iw].bitcast(F32), tmp[:])
                    ln3.stats(hsT, mc, ci, start=(mc == 0), stop=(mc == KD - 1))
        pff.release()

        # ================= Phase 3: LN3 + cross-attention (bf16) =================
        pln3 = tc.alloc_tile_pool(name="pln3", bufs=1, side="right")
        ln3T = pln3.tile([128, KD, NV], BF16, tag="ln3T")
        ln3.finish(hsT, ln3T, norm_t[:, 4, :], norm_t[:, 5, :], ones_bc, eps_t)
        res.release()  # hsT dead

        pp3 = tc.alloc_tile_pool(name="pp3", bufs=2, space="PSUM")
        pq3 = tc.alloc_tile_pool(name="pq3", bufs=1)
        qTa = pq3.tile([128, KD, NV], BF16, tag="qTa")
        with tc.tile_pool(name="pwqa", bufs=2) as pwqa:
            for mc in range(KD):
                if mc % 2 == 0:
                    wqa = pwqa.tile([128, KD, 256], BF16, tag="wqa")
                    nc.sync.dma_start(out=wqa[:],
                                      in_=r128(d_wqa[:, mc * 128:(mc + 2) * 128]))
                mo = (mc % 2) * 128
                for (io, iw) in IC_NV:
                    ps = pp3.tile([128, iw], F32, tag="ps_p3")
                    for kc in range(KD):
                        nc.tensor.matmul(ps[:], wqa[:, kc, mo:mo + 128],
                                         ln3T[:, kc, io:io + iw],
                                         start=(kc == 0), stop=(kc == KD - 1))
                    nc.scalar.activation(qTa[:, mc, io:io + iw], ps[:], AF.Copy)
        pln3.release()

        penc = tc.alloc_tile_pool(name="penc", bufs=1)
        enc_sb = penc.tile([128, KT, LTP], BF16, tag="enc_sb")
        nc.sync.dma_start(out=enc_sb[:], in_=r128(d_encT))
        kTaz = penc.tile([128, HA, LTP], BF16, tag="kTaz")
        nc.vector.memset(kTaz[:], 0.0)
        v1az = penc.tile([LTP, HA, 128], BF16, tag="v1az")
        nc.vector.memset(v1az[:, :, CA + 1:128], 0.0)
        # ones column; 0 in the padded key row so it drops out of the denominator
        nc.sync.dma_start(out=v1az[:, :, CA:CA + 1], in_=d_vones.unsqueeze(2))
        with tc.tile_pool(name="pwenc", bufs=1) as pwenc:
            wka = pwenc.tile([128, KT, INNER_A], BF16, tag="w_enc")
            nc.sync.dma_start(out=wka[:], in_=r128(d_wka))
            for mc in range(KD):
                ps = pp3.tile([128, LTP], F32, tag="ps_p3")
                for kc in range(KT):
                    nc.tensor.matmul(ps[:], wka[:, kc, mc * 128:(mc + 1) * 128],
                                     enc_sb[:, kc, :], start=(kc == 0), stop=(kc == KT - 1))
                nc.scalar.activation(kTaz[0:64, 2 * mc, :], ps[0:64, :], AF.Copy)
                nc.scalar.activation(kTaz[64:128, 2 * mc + 1, :], ps[64:128, :],
                                     AF.Copy)
            wva = pwenc.tile([128, KT, INNER_A], BF16, tag="w_enc")
            nc.sync.dma_start(out=wva[:], in_=r128(d_wva))
            for (co, cw) in DC_D:
                ps = pp3.tile([LTP, cw], F32, tag="ps_p3")
                for kc in range(KT):
                    nc.tensor.matmul(ps[:], enc_sb[:, kc, :], wva[:, kc, co:co + cw],
                                     start=(kc == 0), stop=(kc == KT - 1))
                nc.scalar.activation(v1az[:, co // CA:(co + cw) // CA, 0:CA],
                                     ps[:].rearrange("p (h c) -> p h c", c=CA), AF.Copy)

        pcat3 = tc.alloc_tile_pool(name="pcat3", bufs=1, side="right")
        catTa = pcat3.tile([128, KD, NV], BF16, tag="catTa")
        pp3.release()
        pdena = tc.alloc_tile_pool(name="pdena", bufs=1)
        densa = pdena.tile([HA, NV], F32, tag="densa")
        dena0 = [pdena.tile([1, NV], F32, tag=f"dena0_{h}", name=f"dena0_{h}")
                 for h in range(HA)]
        with (
            tc.tile_pool(name="p3s", bufs=3) as p3s,
            tc.tile_pool(name="psa", bufs=3, space="PSUM") as psa,
            tc.tile_pool(name="pwarm3", bufs=1, space="PSUM") as pwarm3,
        ):
            warm3_ps = pwarm3.tile([128, 512], F32, tag="warm3_ps")

            def warm3(n=3):
                for _ in range(n):
                    nc.tensor.matmul(warm3_ps[:, 0:256], ones_b16[:, 0:128],
                                     ones_b16[:, :], start=True, stop=True,
                                     skip_group_check=True)
            # lookahead-1 pipeline, same reasoning as phase 1
            def emit_sim3(h, i):
                pr, hc = (h % 2) * 64, h // 2
                io, iw = IC_NV[i]
                if i == 0:
                    emit_sim3.pta = p3s.tile([LTP, NV], BF16, tag="pta")
                ps_s = psa.tile([LTP, iw], F32, tag="ps_sa")
                nc.tensor.matmul(ps_s[:], kTaz[:, h, :],
                                 qTa[:, hc, io:io + iw],
                                 start=True, stop=True)
                nc.scalar.activation(emit_sim3.pta[:, io:io + iw], ps_s[:],
                                     AF.Exp, scale=SCALE)
                return (h, i, emit_sim3.pta)

            def emit_av3(st):
                h, i, pta = st
                pr, hc = (h % 2) * 64, h // 2
                io, iw = IC_NV[i]
                if i == 0:
                    emit_av3.dena_h = p3s.tile([1, NV], F32, tag="dena_h")
                ps_av = psa.tile([128, iw], F32, tag="ps_ava")
                nc.tensor.matmul(ps_av[:], v1az[:, h, :], pta[:, io:io + iw],
                                 start=True, stop=True)
                nc.vector.tensor_copy(catTa[pr:pr + 64, hc, io:io + iw],
                                      ps_av[0:CA, :])
                nc.scalar.activation(emit_av3.dena_h[:, io:io + iw],
                                     ps_av[CA:CA + 1, :], AF.Copy)
                if i == len(IC_NV) - 1:
                    nc.sync.dma_start(out=densa[h:h + 1, :], in_=emit_av3.dena_h[:])

            units3 = [(h, i) for h in range(HA) for i in range(len(IC_NV))]
            from collections import deque as _dq
            pend3 = _dq()
            for (h, i) in units3:
                pend3.append(emit_sim3(h, i))
                warm3(3)
                if len(pend3) > 2:
                    emit_av3(pend3.popleft())
            while pend3:
                emit_av3(pend3.popleft())
        nc.vector.reciprocal(densa[:], densa[:])
        with tc.tile_pool(name="pdbca", bufs=2, space="PSUM") as pdbca:
            for h in range(HA):
                pr = (h % 2) * 64
                hc = h // 2
                nc.sync.dma_start(out=dena0[h][:], in_=densa[h:h + 1, :])
                for i, (io, iw) in enumerate(IC_NV):
                    pd = pdbca.tile([128, iw], F32, tag=f"den_bca{i}",
                                    name=f"den_bca{i}")
                    nc.tensor.matmul(pd[:], ones_bc[:],
                                     dena0[h][:, io:io + iw].bitcast(F32R),
                                     start=True, stop=True)
                    nc.vector.tensor_mul(catTa[pr:pr + 64, hc, io:io + iw],
                                         catTa[pr:pr + 64, hc, io:io + iw],
                                         pd[pr:pr + 64, :])
        pdena.release()
        penc.release()
        pq3.release()

        # Final projection, token-major out (stationary = catTa chunks).
        with (
            tc.tile_pool(name="pwoa", bufs=2) as pwoa,
            tc.tile_pool(name="po", bufs=3) as po,
            tc.tile_pool(name="poo", bufs=2, space="PSUM") as poo,
        ):
            for (dco, dcw) in DC_D:
                woa = pwoa.tile([128, KD, dcw], BF16, tag="woa")
                nc.sync.dma_start(out=woa[:], in_=r128(d_woa[:, dco:dco + dcw]))
                for it in range(NV // 128):
                    ps = poo.tile([128, dcw], F32, tag="ps_oo")
                    for kc in range(KD):
                        nc.tensor.matmul(ps[:], catTa[:, kc, it * 128:(it + 1) * 128],
                                         woa[:, kc, :], start=(kc == 0), stop=(kc == KD - 1))
                    o_sb = po.tile([128, dcw], F32, tag="o_sb")
                    nc.vector.tensor_add(o_sb[:], ps[:], boa_b[:, dco:dco + dcw])
                    nc.sync.dma_start(out=d_out[it * 128:(it + 1) * 128, dco:dco + dcw],
                                      in_=o_sb[:])
        pcat3.release()
        cst.release()

    nc.compile()
    return nc


_CACHE = {}


def _get_nc():
    if "nc" not in _CACHE:
        _CACHE["nc"] = build_nc()
    return _CACHE["nc"]


def prepare_in_maps(inputs):
    f32 = np.float32
    bf16 = ml_dtypes.bfloat16
    fp8 = ml_dtypes.float8_e4m3
    hidT = np.ascontiguousarray(inputs["hidden_states"].transpose(0, 2, 1), dtype=f32)
    objT = np.ascontiguousarray(inputs["object_embeddings"].transpose(0, 2, 1), dtype=f32)
    encT = np.zeros((B, DTXT, LTP), dtype=f32)
    encT[:, :, :LTXT] = inputs["encoder_hidden_states"].transpose(0, 2, 1)
    masks = inputs["object_attention_masks"]
    maskT = np.zeros((B, HC, NP, NP), dtype=ml_dtypes.bfloat16)
    maskT[:, :, :N, :N] = (masks.transpose(0, 1, 3, 2) > 0)
    maskT[:, :, 0, N:NP] = 1  # keep padded-query denominators nonzero

    ta = float(np.tanh(inputs["alpha_attn"]))
    td = float(np.tanh(inputs["alpha_dense"]))
    w_co = (np.asarray(inputs["cma_out_w"]) * ta).astype(bf16)
    b_co = (np.asarray(inputs["cma_out_b"]) * ta).astype(f32)
    w_f = (np.asarray(inputs["ffout_w"]) * (td * WS)).astype(fp8)
    b_f = (np.asarray(inputs["ffout_b"]) * td).astype(f32)
    w_g = (np.asarray(inputs["geglu_w"]) * WS).astype(fp8)
    b_g = np.asarray(inputs["geglu_b"]).astype(f32).copy()
    b_g[:DFF] *= FS  # a-half bias pre-scaled (ffT stored as FS*a*gelu)
    norms = np.stack([inputs["norm1_g"], inputs["norm1_b"], inputs["norm2_g"],
                      inputs["norm2_b"], inputs["norm3_g"], inputs["norm3_b"]]).astype(f32)
    shared = {
        "w_lin": np.ascontiguousarray(inputs["linear_w"], dtype=f32),
        "b_lin": np.ascontiguousarray(inputs["linear_b"], dtype=f32),
        "w_q": (np.asarray(inputs["cma_q_w"]) * WS).astype(fp8),
        "w_k": (np.asarray(inputs["cma_k_w"]) * WS).astype(fp8),
        "w_v": (np.asarray(inputs["cma_v_w"]) * WS).astype(fp8),
        "w_co": w_co, "b_co": b_co,
        "w_geglu": w_g, "b_geglu": b_g,
        "w_ffout": w_f, "b_ffout": b_f,
        "w_qa": np.asarray(inputs["attn_q_w"]).astype(bf16),
        "w_ka": np.asarray(inputs["attn_k_w"]).astype(bf16),
        "w_va": np.asarray(inputs["attn_v_w"]).astype(bf16),
        "w_oa": np.asarray(inputs["attn_out_w"]).astype(bf16),
        "b_oa": np.ascontiguousarray(inputs["attn_out_b"], dtype=f32),
        "norms": norms,
        "vones": np.concatenate([np.ones((LTXT, HA), f32),
                                 np.zeros((LTP - LTXT, HA), f32)], axis=0).astype(bf16),
    }
    in_maps = []
    for b in range(B):
        m = dict(shared)
        m["hidT"] = hidT[b]
        m["objT"] = objT[b]
        m["encT"] = encT[b].astype(bf16)
        m["maskT"] = np.ascontiguousarray(maskT[b])
        in_maps.append(m)
    return in_maps


def run(inputs, trace=False):
    nc = _get_nc()
    in_maps = prepare_in_maps(inputs)
    res = run_bass_kernel_spmd(nc, in_maps, core_ids=list(range(B)), trace=trace)
    out = np.stack([res.results[b]["out"] for b in range(B)], axis=0)
    return out, res


def kernel(**inputs):
    out, _ = run(inputs, trace=False)
    return out


# revision 28
# speedup vs baseline: 1.0378x; 1.0378x over previous
"""Trainium2 Bass kernel for nn_CustomAttnProcessor (dense transformer block).

Data-parallel over batch B=8 across 8 NeuronCores; one batch element per core.

Per-core dataflow (channel-major activations: [feature_partition, token_free]):
  xT = concat(hiddenT, obj @ linear_w)            [1280, 1056pad]
  ln1T = LN(xT) -> fp8                            -> masked self-attention
      QKV projections in fp8 DoubleRow (2 k-planes per matmul);
      sim computed TRANSPOSED ([key, query]) so softmax probs are directly
      usable as the moving operand of the attn@V matmul; no max-subtraction
      (values are small), denominator via an appended ones-column on V.
  hsT = hiddenT + tanh(a_attn)*attn[:1024]        (tanh folded into weights)
  hsT += tanh(a_dense)*GEGLU_FFN(LN(hsT))         (fp8 DoubleRow FFN)
  out = cross_attention(LN(hsT), enc)             (bf16) token-major output
"""

import os
import sys

import numpy as np
import ml_dtypes

sys.path.insert(0, "/opt/trn_rl_repo")

import concourse.bass as bass
import concourse.tile as tile
from concourse import bacc, mybir
from concourse.bass_utils import run_bass_kernel_spmd

F32 = mybir.dt.float32
F32R = mybir.dt.float32r
BF16 = mybir.dt.bfloat16
FP8 = mybir.dt.float8e4
U8 = mybir.dt.uint8
AF = mybir.ActivationFunctionType
ALU = mybir.AluOpType
DR = mybir.MatmulPerfMode.DoubleRow

B = 8
NV = 1024          # visual tokens
NOBJ = 30
N = NV + NOBJ      # 1054
NP = 1056          # padded token count (free dim)
NJC = 9            # key-dim 128-chunks over NP (last chunk = 32 rows)
D = 1280
KD = D // 128      # 10
DTXT = 768
KT = DTXT // 128   # 6
LTXT = 77
LTP = 78           # padded (fp32r needs even moving dims)
HC, CC = 8, 64     # masked self-attention heads
HA, CA = 20, 64    # cross-attention heads
INNER_C = HC * CC  # 512
INNER_A = HA * CA  # 1280
DFF = 4 * D        # 5120
KF = DFF // 128    # 40
EPS = 1e-5
SCALE = CC ** -0.5  # 0.125
WS = 32.0          # fp8 weight pre-scale (host); 1/WS folded into psum drain
FS = 8.0           # ffT (a*gelu) fp8 pre-scale

IC_NP = [(0, 512), (512, 512), (1024, 32)]   # token chunks for 1056
IC_NV = [(0, 512), (512, 512)]               # token chunks for 1024
DC_D = [(0, 512), (512, 512), (1024, 256)]   # feature chunks for 1280


class LNPipe:
    """LN over the partition (feature) axis of channel-major x, split so the
    stats matmuls can be emitted inline with the producer of x (keeping the
    PE busy across phase transitions).

    Stats via ones-matmul (cross-partition reduce); mean/rstd rows broadcast
    back across partitions with a rank-1 (K=1) ones matmul into PSUM.
    """

    def __init__(self, tc, nc, name, n_tok, kc_n, ones_r):
        self.tc, self.nc, self.n_tok, self.kc_n = tc, nc, n_tok, kc_n
        self.ones_r = ones_r
        self.chunks = [(o, min(w, n_tok - o)) for (o, w) in IC_NP if o < n_tok]
        # alloc order matters: released LIFO at the end of finish()
        self.stat_ps = tc.alloc_tile_pool(name=f"{name}_st", bufs=1, space="PSUM")
        self.sqp = tc.alloc_tile_pool(name=f"{name}_sq", bufs=2)
        self.rows = tc.alloc_tile_pool(name=f"{name}_rows", bufs=1)
        self.bcast = tc.alloc_tile_pool(name=f"{name}_bc", bufs=1)
        self.tmps = tc.alloc_tile_pool(name=f"{name}_tmp", bufs=2)
        self.ps_mu = [self.stat_ps.tile([1, w], F32, tag=f"ps_mu{i}",
                                        name=f"{name}_mu{i}")
                      for i, (o, w) in enumerate(self.chunks)]
        self.ps_ex = [self.stat_ps.tile([1, w], F32, tag=f"ps_ex{i}",
                                        name=f"{name}_ex{i}")
                      for i, (o, w) in enumerate(self.chunks)]

    def stats(self, x, kc, ci, start, stop):
        """Emit stats for x[:, kc, chunk ci]."""
        nc = self.nc
        o, w = self.chunks[ci]
        sq = self.sqp.tile([128, w], F32R, tag=f"ln_sq{ci}", name=f"sq{ci}")
        nc.vector.tensor_mul(sq[:], x[:, kc, o:o + w].bitcast(F32),
                             x[:, kc, o:o + w].bitcast(F32))
        nc.tensor.matmul(self.ps_mu[ci][:], self.ones_r[:], x[:, kc, o:o + w],
                         start=start, stop=stop)
        nc.tensor.matmul(self.ps_ex[ci][:], self.ones_r[:], sq[:, :],
                         start=start, stop=stop)

    def finish(self, x, out, g_tile, b_tile, ones_bc, eps_t, d_feat=D):
        tc, nc, n_tok = self.tc, self.nc, self.n_tok
        rows, bcast, tmps = self.rows, self.bcast, self.tmps
        mu_row = rows.tile([1, n_tok], F32R, tag="mu_row")
        ex_row = rows.tile([1, n_tok], F32, tag="ex_row")
        t_row = rows.tile([1, n_tok], F32, tag="t_row")
        var_row = rows.tile([1, n_tok], F32, tag="var_row")
        rs_row = rows.tile([1, n_tok], F32R, tag="rs_row")
        inv_d = 1.0 / float(d_feat)
        for i, (o, w) in enumerate(self.chunks):
            nc.scalar.activation(mu_row[:, o:o + w], self.ps_mu[i][:],
                                 AF.Copy, scale=inv_d)
            nc.scalar.activation(ex_row[:, o:o + w], self.ps_ex[i][:],
                                 AF.Copy, scale=inv_d)
        nc.vector.tensor_mul(t_row[:], mu_row[:].bitcast(F32),
                             mu_row[:].bitcast(F32))
        nc.vector.tensor_sub(var_row[:], ex_row[:], t_row[:])
        self.stat_ps.release()
        nc.scalar.activation(t_row[:], var_row[:], AF.Sqrt, bias=eps_t[:])
        nc.vector.reciprocal(rs_row[:], t_row[:])
        with tc.tile_pool(name="ln_bps", bufs=1, space="PSUM") as bps:
            mu_b = bcast.tile([128, n_tok], F32, tag="mu_b")
            rs_b = bcast.tile([128, n_tok], F32, tag="rs_b")
            for i, (o, w) in enumerate(self.chunks):
                pmu = bps.tile([128, w], F32, tag="pmu", name=f"pmu{i}")
                prs = bps.tile([128, w], F32, tag="prs", name=f"prs{i}")
                nc.tensor.matmul(pmu[:], ones_bc[:], mu_row[:, o:o + w],
                                 start=True, stop=True)
                nc.tensor.matmul(prs[:], ones_bc[:], rs_row[:, o:o + w],
                                 start=True, stop=True)
                nc.scalar.activation(mu_b[:, o:o + w], pmu[:], AF.Copy)
                nc.scalar.activation(rs_b[:, o:o + w], prs[:], AF.Copy)
        for kc in range(self.kc_n):
            t1 = tmps.tile([128, n_tok], F32, tag="ln_t1")
            nc.vector.tensor_sub(t1[:], x[:, kc, :].bitcast(F32), mu_b[:])
            nc.vector.tensor_mul(t1[:], t1[:], rs_b[:])
            nc.vector.tensor_scalar(out=out[:, kc, :], in0=t1[:],
                                    scalar1=g_tile[:, kc:kc + 1],
                                    scalar2=b_tile[:, kc:kc + 1],
                                    op0=ALU.mult, op1=ALU.add)
        self.tmps.release()
        self.bcast.release()
        self.rows.release()
        self.sqp.release()


def build_nc():
    nc = bacc.Bacc("TRN2", target_bir_lowering=False, debug=False, num_devices=B)

    # ---- DRAM I/O (per core) ----
    d_hidT = nc.dram_tensor("hidT", [D, NV], F32R, kind="ExternalInput").ap()
    d_objT = nc.dram_tensor("objT", [DTXT, NOBJ], F32R, kind="ExternalInput").ap()
    d_encT = nc.dram_tensor("encT", [DTXT, LTP], BF16, kind="ExternalInput").ap()
    d_maskT = nc.dram_tensor("maskT", [HC, NP, NP], BF16, kind="ExternalInput").ap()
    d_wlin = nc.dram_tensor("w_lin", [DTXT, D], F32R, kind="ExternalInput").ap()
    d_blin = nc.dram_tensor("b_lin", [D], F32, kind="ExternalInput").ap()
    d_wq = nc.dram_tensor("w_q", [D, INNER_C], FP8, kind="ExternalInput").ap()
    d_wk = nc.dram_tensor("w_k", [D, INNER_C], FP8, kind="ExternalInput").ap()
    d_wv = nc.dram_tensor("w_v", [D, INNER_C], FP8, kind="ExternalInput").ap()
    d_wco = nc.dram_tensor("w_co", [INNER_C, D], BF16, kind="ExternalInput").ap()
    d_bco = nc.dram_tensor("b_co", [D], F32, kind="ExternalInput").ap()
    d_wg = nc.dram_tensor("w_geglu", [D, 2 * DFF], FP8, kind="ExternalInput").ap()
    d_bg = nc.dram_tensor("b_geglu", [2 * DFF], F32, kind="ExternalInput").ap()
    d_wf = nc.dram_tensor("w_ffout", [DFF, D], FP8, kind="ExternalInput").ap()
    d_bf = nc.dram_tensor("b_ffout", [D], F32, kind="ExternalInput").ap()
    d_wqa = nc.dram_tensor("w_qa", [D, INNER_A], BF16, kind="ExternalInput").ap()
    d_wka = nc.dram_tensor("w_ka", [DTXT, INNER_A], BF16, kind="ExternalInput").ap()
    d_wva = nc.dram_tensor("w_va", [DTXT, INNER_A], BF16, kind="ExternalInput").ap()
    d_woa = nc.dram_tensor("w_oa", [INNER_A, D], BF16, kind="ExternalInput").ap()
    d_boa = nc.dram_tensor("b_oa", [D], F32, kind="ExternalInput").ap()
    d_norm = nc.dram_tensor("norms", [6, D], F32, kind="ExternalInput").ap()
    d_vones = nc.dram_tensor("vones", [LTP, HA], BF16, kind="ExternalInput").ap()
    d_out = nc.dram_tensor("out", [NV, D], F32, kind="ExternalOutput").ap()

    r128 = lambda ap: ap.rearrange("(kc p) n -> p kc n", p=128)

    with tile.TileContext(nc) as tc, \
            nc.allow_low_precision(reason="fp8/bf16 rounding is intentional"):
        cst = tc.alloc_tile_pool(name="cst", bufs=1)
        ones_f = cst.tile([128, 128], F32, tag="ones_f")  # memset can't write f32r
        nc.vector.memset(ones_f[:], 1.0)
        ones_r = cst.tile([128, 1], F32R, tag="ones_r")
        nc.vector.tensor_copy(ones_r[:], ones_f[:, 0:1])
        eps_t = cst.tile([1, 1], F32, tag="eps")
        nc.vector.memset(eps_t[:], EPS)
        ones_bc = cst.tile([1, 128], F32R, tag="ones_bc")
        nc.vector.tensor_copy(ones_bc[:], ones_f[0:1, :])
        ones_b16 = cst.tile([128, 256], BF16, tag="ones_b16")
        nc.vector.memset(ones_b16[:], 1.0)
        zeros_f = cst.tile([128, KD, 2], F32, tag="zeros_f")
        nc.vector.memset(zeros_f[:], 0.0)
        norm_t = cst.tile([128, 6, KD], F32, tag="norms")
        nc.sync.dma_start(out=norm_t[:], in_=d_norm.rearrange("g (kc p) -> p g kc", p=128))
        blin_t = cst.tile([128, KD], F32, tag="blin")
        nc.sync.dma_start(out=blin_t[:], in_=d_blin.rearrange("(kc p) -> p kc", p=128))
        bco_t = cst.tile([128, KD], F32, tag="bco")
        nc.sync.dma_start(out=bco_t[:], in_=d_bco.rearrange("(kc p) -> p kc", p=128))
        bg_t = cst.tile([128, 2 * DFF // 128], F32, tag="bg")
        nc.sync.dma_start(out=bg_t[:], in_=d_bg.rearrange("(kc p) -> p kc", p=128))
        bf_t = cst.tile([128, KD], F32, tag="bf")
        nc.sync.dma_start(out=bf_t[:], in_=d_bf.rearrange("(kc p) -> p kc", p=128))
        boa_b = cst.tile([128, D], F32, tag="boa_b")
        nc.sync.dma_start(out=boa_b[:], in_=bass.AP(
            tensor=d_boa.tensor, offset=d_boa.offset, ap=[[0, 128]] + d_boa.ap))

        res = tc.alloc_tile_pool(name="res", bufs=1)  # hsT: lives phases 1-3
        hsT = res.tile([128, KD, NV], F32R, tag="hsT")

        # ================= Phase 1: concat + LN1 + masked self-attention ==========
        pps = tc.alloc_tile_pool(name="pps", bufs=2, space="PSUM")  # drain psums
        px = tc.alloc_tile_pool(name="px", bufs=1)
        xT = px.tile([128, KD, NP], F32R, tag="xT")
        nc.vector.tensor_copy(xT[:, :, N:NP], zeros_f[:])
        ln1 = LNPipe(tc, nc, "ln1", NP, KD, ones_r)
        for kc in range(KD):
            nc.sync.dma_start(out=xT[:, kc, 0:NV], in_=r128(d_hidT)[:, kc, :])
            ln1.stats(xT, kc, 0, start=(kc == 0), stop=(kc == KD - 1))
            ln1.stats(xT, kc, 1, start=(kc == 0), stop=(kc == KD - 1))
        obj_sb = px.tile([128, KT, NOBJ], F32R, tag="obj_sb")
        nc.sync.dma_start(out=obj_sb[:], in_=r128(d_objT))
        with tc.tile_pool(name="pwlin", bufs=1) as pwlin:
            wlin = pwlin.tile([128, KT, D], F32R, tag="wlin")
            nc.sync.dma_start(out=wlin[:], in_=r128(d_wlin))
            for mc in range(KD):
                ps = pps.tile([128, NOBJ], F32, tag="ps_proj")
                for kc in range(KT):
                    nc.tensor.matmul(ps[:], wlin[:, kc, mc * 128:(mc + 1) * 128],
                                     obj_sb[:, kc, :], start=(kc == 0), stop=(kc == KT - 1))
                nc.scalar.activation(xT[:, mc, NV:N], ps[:], AF.Identity,
                                     bias=blin_t[:, mc:mc + 1])
                ln1.stats(xT, mc, 2, start=(mc == 0), stop=(mc == KD - 1))

        # ln1T only feeds QKV; fp8 is enough (attention residual is tanh(0.1)-scaled).
        pln1 = tc.alloc_tile_pool(name="pln1", bufs=1, side="right")
        ln1T = pln1.tile([128, KD, NP], FP8, tag="ln1T")
        ln1.finish(xT, ln1T, norm_t[:, 0, :], norm_t[:, 1, :], ones_bc, eps_t)
        px.release()  # xT dead; residual re-reads hiddenT from DRAM

        pqk = tc.alloc_tile_pool(name="pqk", bufs=1)
        pv1 = tc.alloc_tile_pool(name="pv1", bufs=1)
        qT = pqk.tile([128, 4, NP], BF16, tag="qT")
        # kTz: block-diagonal K — head h's 64 channels live at partitions
        # (h%2)*64, the other 64 partitions are zero, so sim matmuls run as
        # full 128x128 tiles (1 cycle/row) with the full-height q as moving.
        kTz = pqk.tile([128, HC, NP], BF16, tag="kTz")
        nc.vector.memset(kTz[:], 0.0)
        # v1z: V padded to 128 stationary columns: [v(64) | ones | zeros(63)]
        v1z = pv1.tile([128, NJC, HC, 128], BF16, tag="v1z")
        nc.vector.memset(v1z[:, :, :, CC + 1:128], 0.0)
        nc.vector.tensor_copy(v1z[:, :, :, CC],
                              ones_f[:, 0:NJC * HC].rearrange("p (a b) -> p a b", a=NJC))
        with tc.tile_pool(name="pwcma", bufs=2) as pwcma:
            for d_w, dest in ((d_wq, qT), (d_wk, kTz)):
                for mc2 in range(2):
                    w_t = pwcma.tile([128, KD, 256], FP8, tag="w_cma")
                    nc.sync.dma_start(out=w_t[:],
                                      in_=r128(d_w[:, mc2 * 256:(mc2 + 1) * 256]))
                    for mh in range(2):
                        mc = mc2 * 2 + mh
                        for (io, iw) in IC_NP:
                            ps = pps.tile([128, iw], F32, tag="ps_proj")
                            for kc in range(0, KD, 2):
                                nc.tensor.matmul(ps[:],
                                                 w_t[:, kc:kc + 2, mh * 128:(mh + 1) * 128],
                                                 ln1T[:, kc:kc + 2, io:io + iw],
                                                 start=(kc == 0), stop=(kc == KD - 2),
                                                 perf_mode=DR)
                            if dest is qT:
                                nc.scalar.activation(qT[:, mc, io:io + iw], ps[:],
                                                     AF.Copy, scale=1.0 / WS)
                            else:
                                nc.scalar.activation(kTz[0:64, 2 * mc, io:io + iw],
                                                     ps[0:64, :], AF.Copy,
                                                     scale=1.0 / WS)
                                nc.scalar.activation(kTz[64:128, 2 * mc + 1, io:io + iw],
                                                     ps[64:128, :], AF.Copy,
                                                     scale=1.0 / WS)
            for mc2 in range(2):
                w_t = pwcma.tile([128, KD, 256], FP8, tag="w_cma")
                nc.sync.dma_start(out=w_t[:], in_=r128(d_wv[:, mc2 * 256:(mc2 + 1) * 256]))
                for jc in range(NJC):
                    jw = 128 if jc < NJC - 1 else NP - 128 * (NJC - 1)
                    ps = pps.tile([128, 256], F32, tag="ps_proj")
                    for kc in range(0, KD, 2):
                        nc.tensor.matmul(ps[:jw, :],
                                         ln1T[:, kc:kc + 2, jc * 128:jc * 128 + jw],
                                         w_t[:, kc:kc + 2, :],
                                         start=(kc == 0), stop=(kc == KD - 2),
                                         perf_mode=DR)
                    nc.scalar.activation(v1z[:jw, jc, mc2 * 4:(mc2 + 1) * 4, 0:CC],
                                         ps[:jw, :].rearrange("p (h c) -> p h c", c=CC),
                                         AF.Copy, scale=1.0 / WS)
        pln1.release()
        pps.release()

        # Attention: simT[j,i] per head; P^T = exp(simT)*maskT; AV via ones-col on V.
        pcat = tc.alloc_tile_pool(name="pcat", bufs=1, side="right")
        catT = pcat.tile([128, 4, NP], BF16, tag="catT")
        pden = tc.alloc_tile_pool(name="pden", bufs=1)
        dens = pden.tile([HC, NP], F32, tag="dens")
        den0 = [pden.tile([1, NP], F32, tag=f"den0_{h}", name=f"den0_{h}")
                for h in range(HC)]
        NMASK = HC * NJC
        MPF = 6  # mask DMA prefetch depth
        with (
            tc.tile_pool(name="p1s", bufs=3) as p1s,
            tc.tile_pool(name="pm8", bufs=MPF) as pm8,
            tc.tile_pool(name="psim", bufs=3, space="PSUM") as psim,
            tc.tile_pool(name="pav", bufs=1, space="PSUM") as pav,
            tc.tile_pool(name="pwarm", bufs=1, space="PSUM") as pwarm,
        ):
            masks = {}

            def ensure_mask(m_idx):
                if m_idx in masks or m_idx >= NMASK:
                    return
                mh, mjc = divmod(m_idx, NJC)
                mjw = 128 if mjc < NJC - 1 else NP - 128 * (NJC - 1)
                t = pm8.tile([128, NP], BF16, tag="m8")
                nc.sync.dma_start(out=t[:mjw, :],
                                  in_=d_maskT[mh, mjc * 128:mjc * 128 + mjw, :])
                masks[m_idx] = t
            warm_ps = pwarm.tile([128, 512], F32, tag="warm_ps")

            def warm(n=2):
                # always-ready matmuls keep the PE clock ramped (p-state)
                for _ in range(n):
                    nc.tensor.matmul(warm_ps[:, 0:256], ones_b16[:, 0:128],
                                     ones_b16[:, :], start=True, stop=True,
                                     skip_group_check=True)
            # Software-pipelined with lookahead 1: sim(i+1) is emitted BEFORE
            # AV(i) so the in-order PE queue never blocks on the exp->mask
            # chain of unit i while sim work is available.
            av_pools = {}

            def emit_sim(h, jc, i):
                pr, hc = (h % 2) * 64, h // 2
                io, iw = IC_NP[i]
                jw = 128 if jc < NJC - 1 else NP - 128 * (NJC - 1)
                if i == 0:
                    cur = h * NJC + jc
                    for k in range(cur, cur + MPF):
                        ensure_mask(k)
                    emit_sim.m8 = masks[cur]
                if jc == 0 and i == 0:
                    av_pools[h] = [
                        pav.tile([128, w], F32, tag=f"ps_av{k}", name=f"ps_av{k}")
                        for k, (o, w) in enumerate(IC_NP)]
                ps_s = psim.tile([128, iw], F32, tag="ps_sim")
                nc.tensor.matmul(ps_s[:jw, :],
                                 kTz[:, h, jc * 128:jc * 128 + jw],
                                 qT[:, hc, io:io + iw],
                                 start=True, stop=True)
                pt = p1s.tile([128, iw], BF16, tag="pt")
                nc.scalar.activation(pt[:jw, :], ps_s[:jw, :], AF.Exp, scale=SCALE)
                ptm = p1s.tile([128, iw], BF16, tag="ptm")
                nc.vector.tensor_mul(ptm[:jw, :], pt[:jw, :],
                                     emit_sim.m8[:jw, io:io + iw])
                return (h, jc, i, jw, ptm)

            def emit_av(st):
                h, jc, i, jw, ptm = st
                pr, hc = (h % 2) * 64, h // 2
                nc.tensor.matmul(av_pools[h][i][:], v1z[:jw, jc, h, :], ptm[:jw, :],
                                 start=(jc == 0), stop=(jc == NJC - 1))
                if jc == NJC - 1:
                    if i == 0:
                        emit_av.den_h = p1s.tile([1, NP], F32, tag="den_h")
                    io, iw = IC_NP[i]
                    nc.vector.tensor_copy(catT[pr:pr + 64, hc, io:io + iw],
                                          av_pools[h][i][0:CC, :])
                    nc.scalar.activation(emit_av.den_h[:, io:io + iw],
                                         av_pools[h][i][CC:CC + 1, :], AF.Copy)
                    if i == len(IC_NP) - 1:
                        # engines can't write partition h directly; DMA the row
                        nc.sync.dma_start(out=dens[h:h + 1, :], in_=emit_av.den_h[:])

            units = [(h, jc, i) for h in range(HC) for jc in range(NJC)
                     for i in range(len(IC_NP))]
            from collections import deque
            pend = deque()
            for (h, jc, i) in units:
                pend.append(emit_sim(h, jc, i))
                warm(2)
                if len(pend) > 2:
                    emit_av(pend.popleft())
            while pend:
                emit_av(pend.popleft())
        # One batched reciprocal for all heads, then per-head row DMA to
        # partition 0 for the rank-1 broadcast matmul.
        nc.vector.reciprocal(dens[:], dens[:])
        with tc.tile_pool(name="pdbc", bufs=2, space="PSUM") as pdbc:
            for h in range(HC):
                pr = (h % 2) * 64
                hc = h // 2
                nc.sync.dma_start(out=den0[h][:], in_=dens[h:h + 1, :])
                for i, (io, iw) in enumerate(IC_NP):
                    pd = pdbc.tile([128, iw], F32, tag=f"den_bc{i}",
                                   name=f"den_bc{i}")
                    nc.tensor.matmul(pd[:], ones_bc[:],
                                     den0[h][:, io:io + iw].bitcast(F32R),
                                     start=True, stop=True)
                    nc.vector.tensor_mul(catT[pr:pr + 64, hc, io:io + iw],
                                         catT[pr:pr + 64, hc, io:io + iw],
                                         pd[pr:pr + 64, :])
        pden.release()
        pv1.release()
        pqk.release()

        # Output projection (tanh(alpha_attn) pre-folded) + residual into hsT.
        # LN2 stats are emitted inline as each hsT chunk lands.
        pln2 = tc.alloc_tile_pool(name="pln2", bufs=1)
        ln2T = pln2.tile([128, KD, NV], FP8, tag="ln2T")
        ln2 = LNPipe(tc, nc, "ln2", NV, KD, ones_r)
        with (
            tc.tile_pool(name="pwco", bufs=1) as pwco,
            tc.tile_pool(name="phid", bufs=3) as phid,
            tc.tile_pool(name="pco", bufs=2, space="PSUM") as pco,
        ):
            w_co = pwco.tile([128, 4, D], BF16, tag="w_co")
            nc.sync.dma_start(out=w_co[:], in_=r128(d_wco))
            for mc in range(KD):
                for ci, (io, iw) in enumerate(IC_NV):
                    ps = pco.tile([128, iw], F32, tag="ps_co")
                    for kc in range(4):
                        nc.tensor.matmul(ps[:], w_co[:, kc, mc * 128:(mc + 1) * 128],
                                         catT[:, kc, io:io + iw],
                                         start=(kc == 0), stop=(kc == 3))
                    hid_r = phid.tile([128, iw], F32R, tag="hid_r")
                    nc.sync.dma_start(out=hid_r[:], in_=r128(d_hidT)[:, mc, io:io + iw])
                    nc.vector.scalar_tensor_tensor(
                        out=hsT[:, mc, io:io + iw], in0=ps[:],
                        scalar=bco_t[:, mc:mc + 1], in1=hid_r[:].bitcast(F32),
                        op0=ALU.add, op1=ALU.add)
                    ln2.stats(hsT, mc, ci, start=(mc == 0), stop=(mc == KD - 1))
        pcat.release()

        # ================= Phase 2: LN2 + GEGLU FFN (fp8 DoubleRow) ==============
        ln2.finish(hsT, ln2T, norm_t[:, 2, :], norm_t[:, 3, :], ones_bc, eps_t)
        pff = tc.alloc_tile_pool(name="pff", bufs=1, side="right")
        ffT = pff.tile([128, KF, NV], FP8, tag="ffT")
        with (
            tc.tile_pool(name="pwg", bufs=2) as pwg,
            tc.tile_pool(name="p2s", bufs=3) as p2s,
            tc.tile_pool(name="p2ps", bufs=2, space="PSUM") as p2ps,
        ):
            for m in range(KF):
                if m % 2 == 0:
                    wga = pwg.tile([128, KD, 256], FP8, tag="wga")
                    nc.sync.dma_start(out=wga[:], in_=r128(d_wg[:, m * 128:(m + 2) * 128]))
                    wgg = pwg.tile([128, KD, 256], FP8, tag="wgg")
                    nc.sync.dma_start(out=wgg[:],
                                      in_=r128(d_wg[:, DFF + m * 128:DFF + (m + 2) * 128]))
                mo = (m % 2) * 128
                gelu_sb = p2s.tile([128, NV], BF16, tag="gelu_sb")
                a_sb = p2s.tile([128, NV], BF16, tag="a_sb")
                for i, (io, iw) in enumerate(IC_NV):
                    ps_a = p2ps.tile([128, iw], F32, tag=f"ps_a{i}", name=f"ps_a{i}")
                    ps_g = p2ps.tile([128, iw], F32, tag=f"ps_g{i}", name=f"ps_g{i}")
                    for kc in range(0, KD, 2):
                        nc.tensor.matmul(ps_a[:], wga[:, kc:kc + 2, mo:mo + 128],
                                         ln2T[:, kc:kc + 2, io:io + iw],
                                         start=(kc == 0), stop=(kc == KD - 2),
                                         perf_mode=DR)
                        nc.tensor.matmul(ps_g[:], wgg[:, kc:kc + 2, mo:mo + 128],
                                         ln2T[:, kc:kc + 2, io:io + iw],
                                         start=(kc == 0), stop=(kc == KD - 2),
                                         perf_mode=DR)
                    # gelu_sb = Gelu(ps_g/WS + bg_gate)
                    nc.scalar.activation(gelu_sb[:, io:io + iw], ps_g[:], AF.Gelu,
                                         bias=bg_t[:, KF + m:KF + m + 1], scale=1.0 / WS)
                    # a_sb = FS*(ps_a/WS + bg_a)  (bias pre-multiplied by FS on host)
                    nc.scalar.activation(a_sb[:, io:io + iw], ps_a[:], AF.Identity,
                                         bias=bg_t[:, m:m + 1], scale=FS / WS)
                    nc.vector.tensor_mul(ffT[:, m, io:io + iw], a_sb[:, io:io + iw],
                                         gelu_sb[:, io:io + iw])
        pln2.release()
        # ffout (tanh(alpha_dense) pre-folded) + residual in place.
        # LN3 stats are emitted inline as each hsT chunk lands.
        ln3 = LNPipe(tc, nc, "ln3", NV, KD, ones_r)
        with (
            tc.tile_pool(name="pwf", bufs=2) as pwf,
            tc.tile_pool(name="pfs", bufs=2) as pfs,
            tc.tile_pool(name="pfps", bufs=3, space="PSUM") as pfps,
        ):
            for mc in range(KD):
                if mc % 2 == 0:
                    wf = pwf.tile([128, KF, 256], FP8, tag="wf")
                    nc.sync.dma_start(out=wf[:], in_=r128(d_wf[:, mc * 128:(mc + 2) * 128]))
                mo = (mc % 2) * 128
                for ci, (io, iw) in enumerate(IC_NV):
                    ps = pfps.tile([128, iw], F32, tag="ps_f")
                    for kc in range(0, KF, 2):
                        nc.tensor.matmul(ps[:], wf[:, kc:kc + 2, mo:mo + 128],
                                         ffT[:, kc:kc + 2, io:io + iw],
                                         start=(kc == 0), stop=(kc == KF - 2),
                                         perf_mode=DR)
                    # tmp = ps/(WS*FS) + bf ; hsT += tmp
                    tmp = pfs.tile([128, iw], F32, tag="fftmp")
                    nc.scalar.activation(tmp[:], ps[:], AF.Identity,
                                         bias=bf_t[:, mc:mc + 1], scale=1.0 / (WS * FS))
                    nc.vector.tensor_add(hsT[:, mc, io:io + iw],
                                         hsT[:, mc, io:io + iw].bitcast(F32), tmp[:])
                    ln3.stats(hsT, mc, ci, start=(mc == 0), stop=(mc == KD - 1))
        pff.release()

        # ================= Phase 3: LN3 + cross-attention (bf16) =================
        pln3 = tc.alloc_tile_pool(name="pln3", bufs=1, side="right")
        ln3T = pln3.tile([128, KD, NV], BF16, tag="ln3T")
        ln3.finish(hsT, ln3T, norm_t[:, 4, :], norm_t[:, 5, :], ones_bc, eps_t)
        res.release()  # hsT dead

        pp3 = tc.alloc_tile_pool(name="pp3", bufs=2, space="PSUM")
        pq3 = tc.alloc_tile_pool(name="pq3", bufs=1)
        qTa = pq3.tile([128, KD, NV], BF16, tag="qTa")
        with tc.tile_pool(name="pwqa", bufs=2) as pwqa:
            for mc in range(KD):
                if mc % 2 == 0:
                    wqa = pwqa.tile([128, KD, 256], BF16, tag="wqa")
                    nc.sync.dma_start(out=wqa[:],
                                      in_=r128(d_wqa[:, mc * 128:(mc + 2) * 128]))
                mo = (mc % 2) * 128
                for (io, iw) in IC_NV:
                    ps = pp3.tile([128, iw], F32, tag="ps_p3")
                    for kc in range(KD):
                        nc.tensor.matmul(ps[:], wqa[:, kc, mo:mo + 128],
                                         ln3T[:, kc, io:io + iw],
                                         start=(kc == 0), stop=(kc == KD - 1))
                    nc.scalar.activation(qTa[:, mc, io:io + iw], ps[:], AF.Copy)
        pln3.release()

        penc = tc.alloc_tile_pool(name="penc", bufs=1)
        enc_sb = penc.tile([128, KT, LTP], BF16, tag="enc_sb")
        nc.sync.dma_start(out=enc_sb[:], in_=r128(d_encT))
        kTaz = penc.tile([128, HA, LTP], BF16, tag="kTaz")
        nc.vector.memset(kTaz[:], 0.0)
        v1az = penc.tile([LTP, HA, 128], BF16, tag="v1az")
        nc.vector.memset(v1az[:, :, CA + 1:128], 0.0)
        # ones column; 0 in the padded key row so it drops out of the denominator
        nc.sync.dma_start(out=v1az[:, :, CA:CA + 1], in_=d_vones.unsqueeze(2))
        with tc.tile_pool(name="pwenc", bufs=1) as pwenc:
            wka = pwenc.tile([128, KT, INNER_A], BF16, tag="w_enc")
            nc.sync.dma_start(out=wka[:], in_=r128(d_wka))
            for mc in range(KD):
                ps = pp3.tile([128, LTP], F32, tag="ps_p3")
                for kc in range(KT):
                    nc.tensor.matmul(ps[:], wka[:, kc, mc * 128:(mc + 1) * 128],
                                     enc_sb[:, kc, :], start=(kc == 0), stop=(kc == KT - 1))
                nc.scalar.activation(kTaz[0:64, 2 * mc, :], ps[0:64, :], AF.Copy)
                nc.scalar.activation(kTaz[64:128, 2 * mc + 1, :], ps[64:128, :],
                                     AF.Copy)
            wva = pwenc.tile([128, KT, INNER_A], BF16, tag="w_enc")
            nc.sync.dma_start(out=wva[:], in_=r128(d_wva))
            for (co, cw) in DC_D:
                ps = pp3.tile([LTP, cw], F32, tag="ps_p3")
                for kc in range(KT):
                    nc.tensor.matmul(ps[:], enc_sb[:, kc, :], wva[:, kc, co:co + cw],
                                     start=(kc == 0), stop=(kc == KT - 1))
                nc.scalar.activation(v1az[:, co // CA:(co + cw) // CA, 0:CA],
                                     ps[:].rearrange("p (h c) -> p h c", c=CA), AF.Copy)

        pcat3 = tc.alloc_tile_pool(name="pcat3", bufs=1, side="right")
        catTa = pcat3.tile([128, KD, NV], BF16, tag="catTa")
        pp3.release()
        pdena = tc.alloc_tile_pool(name="pdena", bufs=1)
        densa = pdena.tile([HA, NV], F32, tag="densa")
        dena0 = [pdena.tile([1, NV], F32, tag=f"dena0_{h}", name=f"dena0_{h}")
                 for h in range(HA)]
        with (
            tc.tile_pool(name="p3s", bufs=3) as p3s,
            tc.tile_pool(name="psa", bufs=3, space="PSUM") as psa,
            tc.tile_pool(name="pwarm3", bufs=1, space="PSUM") as pwarm3,
        ):
            warm3_ps = pwarm3.tile([128, 512], F32, tag="warm3_ps")

            def warm3(n=3):
                for _ in range(n):
                    nc.tensor.matmul(warm3_ps[:, 0:256], ones_b16[:, 0:128],
                                     ones_b16[:, :], start=True, stop=True,
                                     skip_group_check=True)
            # lookahead-1 pipeline, same reasoning as phase 1
            def emit_sim3(h, i):
                pr, hc = (h % 2) * 64, h // 2
                io, iw = IC_NV[i]
                if i == 0:
                    emit_sim3.pta = p3s.tile([LTP, NV], BF16, tag="pta")
                ps_s = psa.tile([LTP, iw], F32, tag="ps_sa")
                nc.tensor.matmul(ps_s[:], kTaz[:, h, :],
                                 qTa[:, hc, io:io + iw],
                                 start=True, stop=True)
                nc.scalar.activation(emit_sim3.pta[:, io:io + iw], ps_s[:],
                                     AF.Exp, scale=SCALE)
                return (h, i, emit_sim3.pta)

            def emit_av3(st):
                h, i, pta = st
                pr, hc = (h % 2) * 64, h // 2
                io, iw = IC_NV[i]
                if i == 0:
                    emit_av3.dena_h = p3s.tile([1, NV], F32, tag="dena_h")
                ps_av = psa.tile([128, iw], F32, tag="ps_ava")
                nc.tensor.matmul(ps_av[:], v1az[:, h, :], pta[:, io:io + iw],
                                 start=True, stop=True)
                nc.vector.tensor_copy(catTa[pr:pr + 64, hc, io:io + iw],
                                      ps_av[0:CA, :])
                nc.scalar.activation(emit_av3.dena_h[:, io:io + iw],
                                     ps_av[CA:CA + 1, :], AF.Copy)
                if i == len(IC_NV) - 1:
                    nc.sync.dma_start(out=densa[h:h + 1, :], in_=emit_av3.dena_h[:])

            units3 = [(h, i) for h in range(HA) for i in range(len(IC_NV))]
            from collections import deque as _dq
            pend3 = _dq()
            for (h, i) in units3:
                pend3.append(emit_sim3(h, i))
                warm3(3)
                if len(pend3) > 2:
                    emit_av3(pend3.popleft())
            while pend3:
                emit_av3(pend3.popleft())
        nc.vector.reciprocal(densa[:], densa[:])
        with tc.tile_pool(name="pdbca", bufs=2, space="PSUM") as pdbca:
            for h in range(HA):
                pr = (h % 2) * 64
                hc = h // 2
                nc.sync.dma_start(out=dena0[h][:], in_=densa[h:h + 1, :])
                for i, (io, iw) in enumerate(IC_NV):
                    pd = pdbca.tile([128, iw], F32, tag=f"den_bca{i}",
                                    name=f"den_bca{i}")
                    nc.tensor.matmul(pd[:], ones_bc[:],
                                     dena0[h][:, io:io + iw].bitcast(F32R),
                                     start=True, stop=True)
                    nc.vector.tensor_mul(catTa[pr:pr + 64, hc, io:io + iw],
                                         catTa[pr:pr + 64, hc, io:io + iw],
                                         pd[pr:pr + 64, :])
        pdena.release()
        penc.release()
        pq3.release()

        # Final projection, token-major out (stationary = catTa chunks).
        with (
            tc.tile_pool(name="pwoa", bufs=2) as pwoa,
            tc.tile_pool(name="po", bufs=3) as po,
            tc.tile_pool(name="poo", bufs=2, space="PSUM") as poo,
        ):
            for (dco, dcw) in DC_D:
                woa = pwoa.tile([128, KD, dcw], BF16, tag="woa")
                nc.sync.dma_start(out=woa[:], in_=r128(d_woa[:, dco:dco + dcw]))
                for it in range(NV // 128):
                    ps = poo.tile([128, dcw], F32, tag="ps_oo")
                    for kc in range(KD):
                        nc.tensor.matmul(ps[:], catTa[:, kc, it * 128:(it + 1) * 128],
                                         woa[:, kc, :], start=(kc == 0), stop=(kc == KD - 1))
                    o_sb = po.tile([128, dcw], F32, tag="o_sb")
                    nc.vector.tensor_add(o_sb[:], ps[:], boa_b[:, dco:dco + dcw])
                    nc.sync.dma_start(out=d_out[it * 128:(it + 1) * 128, dco:dco + dcw],
                                      in_=o_sb[:])
        pcat3.release()
        cst.release()

    nc.compile()
    return nc


_CACHE = {}


def _get_nc():
    if "nc" not in _CACHE:
        _CACHE["nc"] = build_nc()
    return _CACHE["nc"]


def prepare_in_maps(inputs):
    f32 = np.float32
    bf16 = ml_dtypes.bfloat16
    fp8 = ml_dtypes.float8_e4m3
    hidT = np.ascontiguousarray(inputs["hidden_states"].transpose(0, 2, 1), dtype=f32)
    objT = np.ascontiguousarray(inputs["object_embeddings"].transpose(0, 2, 1), dtype=f32)
    encT = np.zeros((B, DTXT, LTP), dtype=f32)
    encT[:, :, :LTXT] = inputs["encoder_hidden_states"].transpose(0, 2, 1)
    masks = inputs["object_attention_masks"]
    maskT = np.zeros((B, HC, NP, NP), dtype=ml_dtypes.bfloat16)
    maskT[:, :, :N, :N] = (masks.transpose(0, 1, 3, 2) > 0)
    maskT[:, :, 0, N:NP] = 1  # keep padded-query denominators nonzero

    ta = float(np.tanh(inputs["alpha_attn"]))
    td = float(np.tanh(inputs["alpha_dense"]))
    w_co = (np.asarray(inputs["cma_out_w"]) * ta).astype(bf16)
    b_co = (np.asarray(inputs["cma_out_b"]) * ta).astype(f32)
    w_f = (np.asarray(inputs["ffout_w"]) * (td * WS)).astype(fp8)
    b_f = (np.asarray(inputs["ffout_b"]) * td).astype(f32)
    w_g = (np.asarray(inputs["geglu_w"]) * WS).astype(fp8)
    b_g = np.asarray(inputs["geglu_b"]).astype(f32).copy()
    b_g[:DFF] *= FS  # a-half bias pre-scaled (ffT stored as FS*a*gelu)
    norms = np.stack([inputs["norm1_g"], inputs["norm1_b"], inputs["norm2_g"],
                      inputs["norm2_b"], inputs["norm3_g"], inputs["norm3_b"]]).astype(f32)
    shared = {
        "w_lin": np.ascontiguousarray(inputs["linear_w"], dtype=f32),
        "b_lin": np.ascontiguousarray(inputs["linear_b"], dtype=f32),
        "w_q": (np.asarray(inputs["cma_q_w"]) * WS).astype(fp8),
        "w_k": (np.asarray(inputs["cma_k_w"]) * WS).astype(fp8),
        "w_v": (np.asarray(inputs["cma_v_w"]) * WS).astype(fp8),
        "w_co": w_co, "b_co": b_co,
        "w_geglu": w_g, "b_geglu": b_g,
        "w_ffout": w_f, "b_ffout": b_f,
        "w_qa": np.asarray(inputs["attn_q_w"]).astype(bf16),
        "w_ka": np.asarray(inputs["attn_k_w"]).astype(bf16),
        "w_va": np.asarray(inputs["attn_v_w"]).astype(bf16),
        "w_oa": np.asarray(inputs["attn_out_w"]).astype(bf16),
        "b_oa": np.ascontiguousarray(inputs["attn_out_b"], dtype=f32),
        "norms": norms,
        "vones": np.concatenate([np.ones((LTXT, HA), f32),
                                 np.zeros((LTP - LTXT, HA), f32)], axis=0).astype(bf16),
    }
    in_maps = []
    for b in range(B):
        m = dict(shared)
        m["hidT"] = hidT[b]
        m["objT"] = objT[b]
        m["encT"] = encT[b].astype(bf16)
        m["maskT"] = np.ascontiguousarray(maskT[b])
        in_maps.append(m)
    return in_maps


def run(inputs, trace=False):
    nc = _get_nc()
    in_maps = prepare_in_maps(inputs)
    res = run_bass_kernel_spmd(nc, in_maps, core_ids=list(range(B)), trace=trace)
    out = np.stack([res.results[b]["out"] for b in range(B)], axis=0)
    return out, res


def kernel(**inputs):
    out, _ = run(inputs, trace=False)
    return out
